# revision 11
# baseline (speedup 1.0000x reference)
"""Complex LSTM layer (B=32, S=512, I=H=512) on 8 trn2 NeuronCores.

Strategy:
  - Phase 1 (sharded over timesteps): each core computes the input-side
    projections x_t = [z_r | z_i] @ W_zbig + bias for its S/8 timesteps as one
    big GEMM, writes them to DRAM, then an AllGather replicates the full
    projection buffer to every core.
  - Phase 2 (replicated): every core runs the full sequential recurrence for
    all 32 batch rows.  Per-step cost on the PE is independent of M (batch),
    so replicating costs no wall-clock vs. batch-sharding and avoids any
    per-step collectives (which have a ~5us floor and would dominate).
    Each core DMAs out the hidden-sequence slice for its 4 batch rows
    (via the transposed h tiles, free-dim dynamic slicing by partition id).

Weights are packed on the host into combined matrices:
  column blocks (512 each, order): [ct_r, ct_i, gate_i, gate_f, gate_o]
  W_hbig rows: [h_r (512); h_i (512)],  W_zbig rows: [z_r; z_i]
so each step's 10 logical matmuls become one (32,1024)@(1024,2560) GEMM.
"""

import sys

sys.path.insert(0, "/opt/trn_rl_repo")

import numpy as np

import concourse.bass as bass
import concourse.bacc as bacc
import concourse.mybir as mybir
import concourse.tile as tile
from concourse import masks
from concourse.bass_utils import run_bass_kernel_spmd

F32 = mybir.dt.float32
AF = mybir.ActivationFunctionType
ALU = mybir.AluOpType

B, I, H = 32, 512, 512
K2 = 2 * H       # 1024 contraction dim (h_r ; h_i)
N5 = 5 * H       # 2560 combined gate columns
NK = K2 // 128   # 8 K-chunks
# column block offsets
CR, CI, GI, GF, GO = 0, H, 2 * H, 3 * H, 4 * H
EPS = 1e-14


def _pack_weights(inp):
    """Build W_zbig [2I, 5H], W_hbig [2H, 5H], bias [5H] from the 26 inputs."""

    def col(zr, zi):  # rows [real; imag] contributions
        return np.concatenate([zr, zi], axis=0)

    wz = np.concatenate(
        [
            col(inp["W_zc_real"], -inp["W_zc_imag"]),
            col(inp["W_zc_imag"], inp["W_zc_real"]),
            col(inp["W_zi_real"], -inp["W_zi_imag"]),
            col(inp["W_zf_real"], -inp["W_zf_imag"]),
            col(inp["W_zo_real"], -inp["W_zo_imag"]),
        ],
        axis=1,
    ).astype(np.float32)
    wh = np.concatenate(
        [
            col(inp["W_hc_real"], -inp["W_hc_imag"]),
            col(inp["W_hc_imag"], inp["W_hc_real"]),
            col(inp["W_hi_real"], -inp["W_hi_imag"]),
            col(inp["W_hf_real"], -inp["W_hf_imag"]),
            col(inp["W_ho_real"], -inp["W_ho_imag"]),
        ],
        axis=1,
    ).astype(np.float32)
    bias = np.concatenate(
        [
            inp["b_c_real"],
            inp["b_c_imag"],
            inp["b_i_real"],
            inp["b_f_real"],
            inp["b_o_real"],
        ]
    ).astype(np.float32)
    return wz, wh, bias


def build_program(S, ncores, unroll):
    """Build the SPMD Bass program (identical on all cores)."""
    TPC = S // ncores  # timesteps computed per core in phase 1
    assert TPC % 4 == 0 and S % unroll == 0

    nc = bacc.Bacc(
        "TRN2", target_bir_lowering=False, debug=False, num_devices=ncores
    )

    # z inputs are host-pre-permuted: [TPC//4, 128, I] with rows p = 32*tt + b
    z_r = nc.declare_dram_parameter("z_r", [TPC // 4, 128, I], F32, isOutput=False)
    z_i = nc.declare_dram_parameter("z_i", [TPC // 4, 128, I], F32, isOutput=False)
    wz_d = nc.declare_dram_parameter("wz", [K2, N5], F32, isOutput=False)
    wh_d = nc.declare_dram_parameter("wh", [K2, N5], F32, isOutput=False)
    bias_d = nc.declare_dram_parameter("brep", [128, N5], F32, isOutput=False)

    hseq_r = nc.declare_dram_parameter("hseq_r", [B, S, H], F32, isOutput=True)
    hseq_i = nc.declare_dram_parameter("hseq_i", [B, S, H], F32, isOutput=True)
    hf_r = nc.declare_dram_parameter("hf_r", [B, H], F32, isOutput=True)
    hf_i = nc.declare_dram_parameter("hf_i", [B, H], F32, isOutput=True)
    cf_r = nc.declare_dram_parameter("cf_r", [B, H], F32, isOutput=True)
    cf_i = nc.declare_dram_parameter("cf_i", [B, H], F32, isOutput=True)

    # internal DRAM: local x projections and the all-gathered full buffer
    xloc = nc.dram_tensor("xloc", [TPC * B, N5], F32)
    xbuf = nc.dram_tensor("xbuf", [S * B, N5], F32, addr_space="Shared")

    with tile.TileContext(nc) as tc:
        from contextlib import ExitStack

        with ExitStack() as ctx:
            wpool = ctx.enter_context(tc.tile_pool(name="w", bufs=1))
            state = ctx.enter_context(tc.tile_pool(name="state", bufs=1))

            id128 = wpool.tile([128, 128], F32, tag="id128")
            masks.make_identity(nc, id128[:])
            id32 = wpool.tile([32, 32], F32, tag="id32")
            masks.make_identity(nc, id32[:])

            bias_sb = wpool.tile([128, N5], F32, tag="bias")
            nc.sync.dma_start(bias_sb[:], bias_d[:])
            eps_sb = wpool.tile([128, 1], F32, tag="eps")
            nc.gpsimd.memset(eps_sb[:], EPS)

            # ---------------- phase 1: x projections (t-sharded) ----------
            wz_sb = []
            for k in range(NK):
                t = wpool.tile([128, N5], F32, tag=f"w{k}", name=f"wz{k}")
                nc.sync.dma_start(t[:], wz_d[128 * k : 128 * (k + 1), :])
                wz_sb.append(t)

            with tc.tile_pool(name="p1sb", bufs=2) as p1sb, tc.tile_pool(
                name="p1ps", bufs=1, space="PSUM"
            ) as p1ps, tc.tile_pool(
                name="p1tr", bufs=2, space="PSUM"
            ) as p1tr, tc.tile_pool(name="p1stg", bufs=2) as p1stg:
                n_mtiles = TPC * B // 128  # rows p = 32*tt + b
                for mt in range(n_mtiles):
                    t0 = mt * 4
                    zr_sb = p1sb.tile([128, I], F32, tag="zr")
                    zi_sb = p1sb.tile([128, I], F32, tag="zi")
                    nc.sync.dma_start(zr_sb[:], z_r[mt])
                    nc.sync.dma_start(zi_sb[:], z_i[mt])
                    xt_sb = []
                    for k in range(NK):
                        src = zr_sb if k < 4 else zi_sb
                        j = k % 4
                        trp = p1tr.tile([128, 128], F32, tag="tr")
                        nc.tensor.transpose(
                            trp[:], src[:, 128 * j : 128 * (j + 1)], id128[:]
                        )
                        xs = p1sb.tile([128, 128], F32, tag=f"xt{k}", name=f"xt{k}")
                        nc.vector.tensor_copy(xs[:], trp[:])
                        xt_sb.append(xs)
                    stage = p1stg.tile([128, N5], F32, tag="stage")
                    for n in range(5):
                        ps = p1ps.tile([128, 512], F32, tag=f"px{n}", name=f"px{n}")
                        for k in range(NK):
                            nc.tensor.matmul(
                                ps[:],
                                xt_sb[k][:],
                                wz_sb[k][:, 512 * n : 512 * (n + 1)],
                                start=(k == 0),
                                stop=(k == NK - 1),
                            )
                        # stage = psum + bias  (psum -> sbuf with fused bias add)
                        nc.vector.scalar_tensor_tensor(
                            stage[:, 512 * n : 512 * (n + 1)],
                            ps[:],
                            1.0,
                            bias_sb[:, 512 * n : 512 * (n + 1)],
                            ALU.mult,
                            ALU.add,
                        )
                    # stage rows (tt, b) == xloc rows (t, b): plain copy
                    nc.sync.dma_start(xloc[B * t0 : B * (t0 + 4), :], stage[:])

            # all-gather the x projections (rank shards stack on axis 0)
            nc.gpsimd.collective_compute(
                "AllGather",
                ALU.bypass,
                ins=[xloc[:]],
                outs=[xbuf[:]],
                replica_groups=[list(range(ncores))],
            )

            # ---------------- phase 2: recurrence (replicated) -------------
            wh_sb = []
            for k in range(NK):
                t = wpool.tile([128, N5], F32, tag=f"w{k}", name=f"wh{k}")  # reuse wz slots
                nc.sync.dma_start(t[:], wh_d[128 * k : 128 * (k + 1), :])
                wh_sb.append(t)

            # persistent state (ping-pong pairs)
            hT = [state.tile([128, 8 * 32], F32, tag=f"hT{p}", name=f"hT{p}") for p in range(2)]
            c_r = [state.tile([B, H], F32, tag=f"cr{p}", name=f"cr{p}") for p in range(2)]
            c_i = [state.tile([B, H], F32, tag=f"ci{p}", name=f"ci{p}") for p in range(2)]
            nc.gpsimd.memset(hT[0][:], 0.0)
            nc.gpsimd.memset(c_r[0][:], 0.0)
            nc.gpsimd.memset(c_i[0][:], 0.0)

            pid = nc.partition_id()
            pid4 = pid * 4

            with tc.tile_pool(name="xt2", bufs=3) as xtp, tc.tile_pool(
                name="ew", bufs=1
            ) as ew, tc.tile_pool(name="hrv", bufs=2) as hrv, tc.tile_pool(
                name="gps", bufs=1, space="PSUM"
            ) as gps, tc.tile_pool(name="trp2", bufs=2, space="PSUM") as trp2:

                def step(t_expr, cur, nxt):
                    xt = xtp.tile([B, N5], F32, tag="xt")
                    nc.sync.dma_start(xt[:], xbuf[bass.ds(t_expr * B, B), :])

                    psg = [gps.tile([B, 512], F32, tag=f"g{g}", name=f"g{g}") for g in range(5)]
                    for g in range(5):
                        for k in range(NK):
                            nc.tensor.matmul(
                                psg[g][:],
                                hT[cur][:, 32 * k : 32 * (k + 1)],
                                wh_sb[k][:, 512 * g : 512 * (g + 1)],
                                start=(k == 0),
                                stop=(k == NK - 1),
                            )

                    def tl(tag):
                        return ew.tile([B, H], F32, tag=tag, name=tag)[:]

                    def xs(g):
                        return xt[:, 512 * g : 512 * (g + 1)]

                    # ct = x_c + pre  (both halves), then complex tanh
                    t_ctr, t_cti = tl("ctr"), tl("cti")
                    nc.vector.scalar_tensor_tensor(
                        t_ctr, psg[0][:], 1.0, xs(0), ALU.mult, ALU.add
                    )
                    nc.vector.scalar_tensor_tensor(
                        t_cti, psg[1][:], 1.0, xs(1), ALU.mult, ALU.add
                    )
                    t_s1, t_s2, t_m2 = tl("s1"), tl("s2"), tl("m2")
                    nc.vector.tensor_mul(t_s1, t_ctr, t_ctr)
                    nc.scalar.square(t_s2, t_cti)
                    nc.vector.tensor_add(t_m2, t_s1, t_s2)
                    t_mag, t_rv, t_th, t_tm = tl("mag"), tl("rv"), tl("th"), tl("tm")
                    nc.scalar.activation(t_mag, t_m2, AF.Sqrt, bias=eps_sb[:B, :])
                    nc.vector.reciprocal(t_rv, t_mag)
                    nc.scalar.activation(t_th, t_mag, AF.Tanh)
                    nc.vector.tensor_mul(t_tm, t_th, t_rv)
                    t_ctr2, t_cti2 = tl("ctr2"), tl("cti2")
                    nc.vector.tensor_mul(t_ctr2, t_tm, t_ctr)
                    nc.gpsimd.tensor_mul(t_cti2, t_tm, t_cti)

                    # gates
                    gates = {}
                    for g, nm in ((2, "i"), (3, "f"), (4, "o")):
                        t_in, t_g = tl(f"in_{nm}"), tl(f"g_{nm}")
                        nc.vector.scalar_tensor_tensor(
                            t_in, psg[g][:], 1.0, xs(g), ALU.mult, ALU.add
                        )
                        nc.scalar.activation(t_g, t_in, AF.Sigmoid)
                        gates[nm] = t_g

                    # c update
                    t_m1r, t_m1i, t_pr, t_pi = tl("m1r"), tl("m1i"), tl("pr"), tl("pi")
                    nc.vector.tensor_mul(t_m1r, gates["f"], c_r[cur][:])
                    nc.gpsimd.tensor_mul(t_m1i, gates["f"], c_i[cur][:])
                    nc.vector.tensor_mul(t_pr, gates["i"], t_ctr2)
                    nc.gpsimd.tensor_mul(t_pi, gates["i"], t_cti2)
                    nc.vector.tensor_add(c_r[nxt][:], t_m1r, t_pr)
                    nc.gpsimd.tensor_add(c_i[nxt][:], t_m1i, t_pi)

                    # output tanh
                    t_s1b, t_s2b, t_m2b = tl("s1b"), tl("s2b"), tl("m2b")
                    nc.vector.tensor_mul(t_s1b, c_r[nxt][:], c_r[nxt][:])
                    nc.scalar.square(t_s2b, c_i[nxt][:])
                    nc.vector.tensor_add(t_m2b, t_s1b, t_s2b)
                    t_magb, t_rvb, t_thb, t_tmb = (
                        tl("magb"),
                        tl("rvb"),
                        tl("thb"),
                        tl("tmb"),
                    )
                    nc.scalar.activation(t_magb, t_m2b, AF.Sqrt, bias=eps_sb[:B, :])
                    nc.vector.reciprocal(t_rvb, t_magb)
                    nc.scalar.activation(t_thb, t_magb, AF.Tanh)
                    nc.vector.tensor_mul(t_tmb, t_thb, t_rvb)
                    t_thr, t_thi = tl("thr"), tl("thi")
                    nc.vector.tensor_mul(t_thr, t_tmb, c_r[nxt][:])
                    nc.gpsimd.tensor_mul(t_thi, t_tmb, c_i[nxt][:])
                    h_r = hrv.tile([B, H], F32, tag="hr")
                    h_i = hrv.tile([B, H], F32, tag="hi")
                    nc.vector.tensor_mul(h_r[:], gates["o"], t_thr)
                    nc.gpsimd.tensor_mul(h_i[:], gates["o"], t_thi)

                    # transpose h into the next stationary [128, (k b)]
                    ptr = trp2.tile([128, 8 * 32], F32, tag="tr")
                    for k in range(8):
                        src = h_r if k < 4 else h_i
                        j = k % 4
                        nc.tensor.transpose(
                            ptr[:, 32 * k : 32 * (k + 1)],
                            src[:, 128 * j : 128 * (j + 1)],
                            id32[:],
                        )
                    nc.vector.tensor_copy(hT[nxt][:], ptr[:])

                    # stream out this core's 4 batches of h_t (from hT views)
                    hTv = hT[nxt].rearrange("p (k b) -> p k b", b=32)
                    for name, dst in (("r", hseq_r), ("i", hseq_i)):
                        koff = 0 if name == "r" else 4
                        dflat = dst.rearrange("b s h -> b (s h)")
                        for k in range(4):
                            nc.sync.dma_start(
                                dflat[
                                    bass.ds(pid4, 4),
                                    bass.ds(t_expr * H + 128 * k, 128),
                                ].rearrange("b h2 -> h2 b"),
                                hTv[:, koff + k, bass.ds(pid4, 4)],
                            )
                    return h_r, h_i

                with tc.For_i(
                    0,
                    S,
                    unroll,
                    hint_engines=(
                        mybir.EngineType.PE,
                        mybir.EngineType.DVE,
                        mybir.EngineType.Activation,
                        mybir.EngineType.Pool,
                    ),
                ) as iv:
                    for u in range(unroll):
                        last = step(iv + u, u % 2, (u + 1) % 2)

                # final states: every core writes the full tensors (identical)
                h_r, h_i = last
                nc.sync.dma_start(hf_r[:], h_r[:])
                nc.sync.dma_start(hf_i[:], h_i[:])
                nc.sync.dma_start(cf_r[:], c_r[0][:])
                nc.sync.dma_start(cf_i[:], c_i[0][:])

    nc.compile()
    return nc


_CACHE = {}


def _get_program(S, ncores, unroll):
    key = (S, ncores, unroll)
    if key not in _CACHE:
        _CACHE[key] = build_program(S, ncores, unroll)
    return _CACHE[key]


def kernel(**inputs):
    S = inputs["z_real"].shape[1]
    ncores = 8
    unroll = 8
    TPC = S // ncores
    wz, wh, bias = _pack_weights(inputs)
    brep = np.broadcast_to(bias, (128, N5)).copy()

    z_r = np.asarray(inputs["z_real"], dtype=np.float32)
    z_i = np.asarray(inputs["z_imag"], dtype=np.float32)

    nc = _get_program(S, ncores, unroll)
    in_maps = []
    for c in range(ncores):
        zr_c = np.ascontiguousarray(
            z_r[:, c * TPC : (c + 1) * TPC, :]
            .transpose(1, 0, 2)
            .reshape(TPC // 4, 128, I)
        )
        zi_c = np.ascontiguousarray(
            z_i[:, c * TPC : (c + 1) * TPC, :]
            .transpose(1, 0, 2)
            .reshape(TPC // 4, 128, I)
        )
        in_maps.append(
            {
                "z_r": zr_c,
                "z_i": zi_c,
                "wz": wz,
                "wh": wh,
                "brep": brep,
            }
        )
    res = run_bass_kernel_spmd(nc, in_maps, list(range(ncores))).results

    hseq_r = np.concatenate(
        [res[c]["hseq_r"][4 * c : 4 * (c + 1)] for c in range(ncores)], axis=0
    )
    hseq_i = np.concatenate(
        [res[c]["hseq_i"][4 * c : 4 * (c + 1)] for c in range(ncores)], axis=0
    )
    hf_r, hf_i = res[0]["hf_r"], res[0]["hf_i"]
    cf_r, cf_i = res[0]["cf_r"], res[0]["cf_i"]
    return ((hseq_r, hseq_i), ((hf_r, hf_i), (cf_r, cf_i)))
